# revision 67
# baseline (speedup 1.0000x reference)
"""Trainium2 Bass kernel for Grossberg dynamics (batched gated 17x17 matvecs).

dS/dt = (-DECAY*s + (B-s)*relu(exc) - (C+s)*relu(inh)) / TAU, masked on actions.

Sharding: pure data-parallel over the agent axis across 8 NeuronCores
(32768 agents per core).

Per-core algorithm (PE-assisted reduce):
  W is host-packed fp16 with the contraction axis j on SBUF partitions:
  partitions = (block b, j) (7 blocks x 17 j = 119 rows main macros);
  free axis = (pair in {pos,neg}, i, agent g). Per macro-tile:
    1. DVE: prod = wt * broadcast(s_t)  -- two tensor_tensor in 2x mode
       (DVE's stream stays almost pure multiply; everything else lives on
       other engines so in-order dispatch never stalls the next mult)
    2. PE : segmented sum over j via block-diagonal-ones matmuls:
       stationary = 128-col chunks of prod [119, 128], moving = indicator
       ones [119, 7]; out[m, b'] = sum_j prod[(b', j), m] -> PSUM fp32.
    3. ACT: drain PSUM -> SBUF fp16 (exin), sigmoid gates, env relus,
       final relu(1.25*x) in place.
    4. Pool: gate/env/lateral RMW on exin, shunting combine, out DMA
       via SWDGE.
  DMA: wst = [st | wpos | wneg] per partition row; first half (st+wpos)
  issues from SP, second half (wneg) from ACT, aux from SP, out from
  Pool SWDGE. Emission is software-pipelined (loads run one macro ahead)
  so each sequencer's in-order stream never blocks next-macro prefetch.

Main macros: A=512 agents/block, 7 blocks = 3584 agents x 9 = 32256.
Tail macro: 4 blocks x 128 agents = 512.  Total 32768 per core.

PSUM mv layout: col = pair*PH + i*(GC*NB) + gc*NB + b, agent-per-partition
index a = (gc, b); PH = 512 main / 256 tail.
"""

import numpy as np

import concourse.bass as bass
import concourse.bacc as bacc
import concourse.mybir as mybir
from concourse.tile import TileContext
from concourse.bass_utils import run_bass_kernel_spmd

P = 128
N = 17
NCORES = 8
B_TOTAL = 262144
B_CORE = B_TOTAL // NCORES  # 32768

# macro geometry list: (nb blocks, agents-per-block). 8 big macros fill the
# DMA-bound steady state; 4 small ones drain the pipeline with short
# epilogues; the 512-agent tail uses 4 blocks of 128.
GEOM = [(7, 512)] * 8 + [(7, 256)] * 2 + [(4, 128)]
NB, A = 7, 512
GC = A // 128
NB_T, A_T, GC_T = 4, 128, 1

FP = mybir.dt.float32
FH = mybir.dt.float16
AX = mybir.AxisListType
OP = mybir.AluOpType
AF = mybir.ActivationFunctionType

# Grossberg constants
TAU, DECAY, B_CAP, C_FLOOR = 0.8, 0.15, 1.0, 0.1
LAT_INHIB, DIV_SIGMA = 3.0, 0.3
ALPHA, BETA = 1.5, 0.75
INV_TAU = 1.0 / TAU                       # 1.25
U_BIAS = DECAY * INV_TAU                  # 0.1875 ; dS = Re - 0.1*Ri - s*(U_BIAS + Re + Ri)
LAT_DEN_C = DIV_SIGMA + 1e-6              # 0.300001

NAUX = 34  # s(17) | pert-needs(9) | pert-valence(4) | feas(4), fp16


def _loads(nc, pools, spec, aux):
    """Issue the W/state DMA for one macro; aux comes from a grouped load."""
    nb, a_blk, (wt_v, aux_v, out_v), ones_tile, sfx, fine = spec
    pool_w, pool_m, pool_s, pool_ps = pools
    nparts = nb * N
    fw = 2 * N * a_blk
    a = a_blk

    wst = pool_w.tile([nparts, a_blk + fw], FH, tag="wt" + sfx, bufs=2 if sfx == "" else 1)
    if fine:
        # 4-way split on i-row boundaries so the mult chunks can chase the
        # transfers during the pipeline drain
        cuts = [0, a + 9 * a, a + 17 * a, a + 26 * a, a + 34 * a]
        for k in range(4):
            eng = nc.sync if k % 2 == 0 else nc.scalar
            eng.dma_start(out=wst[:, cuts[k]:cuts[k + 1]], in_=wt_v[:, cuts[k]:cuts[k + 1]])
    else:
        half = a_blk + fw // 2
        nc.sync.dma_start(out=wst[:, 0:half], in_=wt_v[:, 0:half])
        nc.scalar.dma_start(out=wst[:, half:], in_=wt_v[:, half:])
    return wst, aux


def _front(nc, pools, spec, tiles):
    """Mults + PE reduce + everything derivable from aux alone."""
    nb, a_blk, (wt_v, aux_v, out_v), ones_tile, sfx, fine = spec
    gcn = a_blk // 128
    wst, aux = tiles
    pool_w, pool_m, pool_s, pool_ps = pools
    nparts = nb * N
    fw = 2 * N * a_blk
    napp = gcn * nb
    main = sfx == ""
    PH = 512 if main else 256

    # ---- big multiply (DVE, 2x mode), split pos/neg ----
    st = wst[:, 0:a_blk]
    prod = pool_m.tile([nparts, fw], FH, tag="prod" + sfx, bufs=2 if main else 1)
    s_b = st[:, None, :].broadcast_to([nparts, N, a_blk])
    w4 = wst[:, a_blk:].rearrange("p (t i g) -> p t i g", t=2, i=N)
    p4 = prod.rearrange("p (t i g) -> p t i g", t=2, i=N)
    if fine:
        nc.vector.tensor_tensor(out=p4[:, 0, 0:9], in0=w4[:, 0, 0:9], in1=s_b[:, 0:9], op=OP.mult)
        nc.vector.tensor_tensor(out=p4[:, 0, 9:17], in0=w4[:, 0, 9:17], in1=s_b[:, 9:17], op=OP.mult)
        nc.vector.tensor_tensor(out=p4[:, 1, 0:9], in0=w4[:, 1, 0:9], in1=s_b[:, 0:9], op=OP.mult)
        nc.vector.tensor_tensor(out=p4[:, 1, 9:17], in0=w4[:, 1, 9:17], in1=s_b[:, 9:17], op=OP.mult)
    else:
        nc.vector.tensor_tensor(out=p4[:, 0], in0=w4[:, 0], in1=s_b, op=OP.mult)
        nc.vector.tensor_tensor(out=p4[:, 1], in0=w4[:, 1], in1=s_b, op=OP.mult)

    # ---- segmented j-reduce on the PE ----
    mv = pool_ps.tile([P, 2 * PH], FP, tag="mv" + sfx, bufs=3 if main else 2)
    for t in range(2):
        for i in range(N):
            for gc in range(gcn):
                c = (t * N + i) * gcn + gc
                off = t * PH + (i * gcn + gc) * nb
                nc.tensor.matmul(
                    mv[:, off : off + nb],
                    prod[:, 128 * c : 128 * (c + 1)],
                    ones_tile[:nparts, :nb],
                    start=True,
                    stop=True,
                )

    aux3 = aux.rearrange("p (a c) -> p a c", c=NAUX)
    s_T = aux3[:, :, 0:17]      # [p, a, c] agent-major
    ptn_T = aux3[:, :, 17:26]   # pert needs rows 0:9
    ptv_T = aux3[:, :, 26:30]   # pert valence rows 13:17

    # ---- gates: ve = s_v + p_v (DVE), sigmoids on ACT (transposing) ----
    tb = 2 if main else 1
    ve = pool_s.tile([P, napp * 4], FH, tag="ve" + sfx, bufs=tb)
    ve3 = ve.rearrange("p (a r) -> p a r", r=4)
    nc.vector.tensor_tensor(out=ve3, in0=s_T[:, :, 13:17], in1=ptv_T, op=OP.add)
    veT = ve.rearrange("p (a r) -> p r a", r=4)
    ge = pool_s.tile([P, 4 * napp], FH, tag="ge" + sfx, bufs=tb)
    ge3 = ge.rearrange("p (r a) -> p r a", a=napp)
    nc.scalar.activation(ge3, veT, AF.Sigmoid, scale=ALPHA)
    gi = pool_s.tile([P, 4 * napp], FH, tag="gi" + sfx, bufs=tb)
    gi3 = gi.rearrange("p (r a) -> p r a", a=napp)
    nc.scalar.activation(gi3, veT, AF.Sigmoid, scale=-BETA)

    # ---- env drive relu(+-pert) on needs rows (ACT, transposing) ----
    ptT9 = ptn_T.rearrange("p a i -> p i a")
    rp = pool_s.tile([P, 9 * napp], FH, tag="rp" + sfx, bufs=tb)
    rp3 = rp.rearrange("p (i a) -> p i a", a=napp)
    nc.scalar.activation(rp3, ptT9, AF.Relu)
    rn = pool_s.tile([P, 9 * napp], FH, tag="rn" + sfx, bufs=tb)
    rn3 = rn.rearrange("p (i a) -> p i a", a=napp)
    nc.scalar.activation(rn3, ptT9, AF.Relu, scale=-1.0)

    # ---- lateral inhibition prep (DVE + Pool) ----
    sa = s_T[:, :, 9:13]
    t2 = pool_s.tile([P, napp * 2], FH, tag="t2" + sfx, bufs=tb)
    t23 = t2.rearrange("p (a r) -> p a r", r=2)
    nc.vector.tensor_tensor(out=t23, in0=sa[:, :, 0:2], in1=sa[:, :, 2:4], op=OP.add)
    suma = pool_s.tile([P, napp], FH, tag="suma" + sfx, bufs=tb)
    nc.vector.tensor_tensor(
        out=suma[:, :, None], in0=t23[:, :, 0:1], in1=t23[:, :, 1:2], op=OP.add
    )
    other = pool_s.tile([P, 4 * napp], FH, tag="other" + sfx, bufs=tb)
    other3 = other.rearrange("p (r a) -> p r a", a=napp)
    saT = sa.rearrange("p a r -> p r a")
    nc.vector.tensor_tensor(
        out=other3,
        in0=suma[:, None, :].broadcast_to([P, 4, napp]),
        in1=saT,
        op=OP.subtract,
    )
    den = pool_s.tile([P, 4 * napp], FH, tag="den" + sfx, bufs=tb)
    nc.vector.tensor_scalar(
        out=den[:], in0=other[:], scalar1=LAT_DEN_C, scalar2=1.0 / LAT_INHIB,
        op0=OP.add, op1=OP.mult,
    )
    acr = pool_s.tile([P, 4 * napp], FH, tag="acr" + sfx, bufs=tb)
    nc.vector.reciprocal(acr[:], den[:])
    lat = pool_s.tile([P, 4 * napp], FH, tag="lat" + sfx, bufs=tb)
    nc.gpsimd.tensor_tensor(out=lat[:], in0=other[:], in1=acr[:], op=OP.mult)

    return dict(mv=mv, aux=aux, ge3=ge3, gi3=gi3, rp3=rp3, rn3=rn3, lat=lat)


def _back(nc, pools, spec, ctx, eg=False, eg2=False):
    """mv-dependent epilogue. eg=True splits the chain across DVE+Pool for
    the pipeline-drain macros (DVE is idle there; mid-stream it must stay
    clear of everything that waits on mv)."""
    nb, a_blk, (wt_v, aux_v, out_v), ones_tile, sfx, fine = spec
    gcn = a_blk // 128
    pool_w, pool_m, pool_s, pool_ps = pools
    napp = gcn * nb
    main = sfx == ""
    PH = 512 if main else 256
    nmv = N * napp
    mv, aux = ctx["mv"], ctx["aux"]
    ge3, gi3, rp3, rn3, lat = ctx["ge3"], ctx["gi3"], ctx["rp3"], ctx["rn3"], ctx["lat"]
    lat3 = lat.rearrange("p (r a) -> p r a", a=napp)
    mv3 = mv.rearrange("p (t x) -> p t x", t=2)
    aux3 = aux.rearrange("p (a c) -> p a c", c=NAUX)
    s_T = aux3[:, :, 0:17]
    fs_T = aux3[:, :, 30:34]


    # ---- drain PSUM -> SBUF fp16 ----
    tb = 4 if main else 1
    exin = pool_s.tile([P, 2 * nmv], FH, tag="exin" + sfx, bufs=tb)
    if eg2:
        nc.vector.tensor_copy(out=exin[:, 0:nmv], in_=mv3[:, 0, 0:nmv])
        nc.vector.tensor_copy(out=exin[:, nmv : 2 * nmv], in_=mv3[:, 1, 0:nmv])
    else:
        nc.scalar.activation(exin[:, 0:nmv], mv3[:, 0, 0:nmv], AF.Copy)
        nc.scalar.activation(exin[:, nmv : 2 * nmv], mv3[:, 1, 0:nmv], AF.Copy)
    exc3 = exin[:, 0:nmv].rearrange("p (i a) -> p i a", i=N)
    inh3 = exin[:, nmv : 2 * nmv].rearrange("p (i a) -> p i a", i=N)

    # ---- apply gates / env / lateral ----
    exc_eng = nc.vector if eg else nc.gpsimd
    inh_eng = nc.vector if eg else nc.gpsimd
    exc_eng.tensor_tensor(out=exc3[:, 9:13], in0=exc3[:, 9:13], in1=ge3, op=OP.mult)
    inh_eng.tensor_tensor(out=inh3[:, 9:13], in0=inh3[:, 9:13], in1=gi3, op=OP.mult)
    exc_eng.tensor_tensor(out=exc3[:, 0:9], in0=exc3[:, 0:9], in1=rp3, op=OP.add)
    inh_eng.tensor_tensor(out=inh3[:, 0:9], in0=inh3[:, 0:9], in1=rn3, op=OP.add)
    inh_eng.tensor_tensor(out=inh3[:, 9:13], in0=inh3[:, 9:13], in1=lat3, op=OP.add)

    # ---- shunting combine: dS = Re - 0.1*Ri - s*(U_BIAS + Re + Ri) ----
    re = exin[:, 0:nmv]
    ri = exin[:, nmv : 2 * nmv]
    if eg2:
        nc.vector.tensor_scalar(
            out=re, in0=re, scalar1=INV_TAU, scalar2=0.0, op0=OP.mult, op1=OP.max
        )
        nc.vector.tensor_scalar(
            out=ri, in0=ri, scalar1=INV_TAU, scalar2=0.0, op0=OP.mult, op1=OP.max
        )
    else:
        nc.scalar.activation(re, re, AF.Relu, scale=INV_TAU)
        nc.scalar.activation(ri, ri, AF.Relu, scale=INV_TAU)
    ch = nc.vector if eg else nc.gpsimd
    s1 = pool_s.tile([P, nmv], FH, tag="s1" + sfx, bufs=tb)
    ch.tensor_tensor(out=s1[:], in0=re, in1=ri, op=OP.add)
    sT_i = s_T.rearrange("p a i -> p i a")
    s2 = pool_s.tile([P, nmv], FH, tag="s2" + sfx, bufs=tb)
    if eg:
        # DVE supports fused (in0 op0 scalar) op1 in1 — two fewer chain hops
        nc.vector.scalar_tensor_tensor(
            out=s1[:], in0=s1[:], scalar=U_BIAS, in1=sT_i, op0=OP.add, op1=OP.mult
        )  # = u
        nc.vector.scalar_tensor_tensor(
            out=s2[:], in0=ri, scalar=-C_FLOOR, in1=re, op0=OP.mult, op1=OP.add
        )  # = v
    else:
        ch.tensor_scalar_add(out=s1[:], in0=s1[:], scalar1=U_BIAS)
        ch.tensor_tensor(out=s1[:], in0=s1[:], in1=sT_i, op=OP.mult)  # = u
        ch.tensor_scalar_mul(out=s2[:], in0=ri, scalar1=-C_FLOOR)
        ch.tensor_tensor(out=s2[:], in0=s2[:], in1=re, op=OP.add)     # = v
    ds = ctx["ds"]
    ch.tensor_tensor(out=ds, in0=s2[:], in1=s1[:], op=OP.subtract)
    ds3 = ds.rearrange("p (i a) -> p i a", a=napp)
    fsT = fs_T.rearrange("p a r -> p r a")
    ch.tensor_tensor(out=ds3[:, 9:13], in0=ds3[:, 9:13], in1=fsT, op=OP.mult)


AUXG = [4, 4, 2, 1]      # aux DMAs cover these GEOM runs (same-size macros)
OUTG = [2, 2, 2, 2, 2, 1]  # out DMAs cover these runs


def build_program():
    nc = bacc.Bacc()
    wt_ds = []
    for i, (nb, a) in enumerate(GEOM):
        wt_ds.append(nc.dram_tensor(f"wt{i}", [nb * N, a + 2 * N * a], FH, kind="ExternalInput"))
    aux_ds, gidx = [], 0
    for gi_, cnt in enumerate(AUXG):
        nb, a = GEOM[gidx]
        napp = (a // 128) * nb
        aux_ds.append(nc.dram_tensor(f"auxg{gi_}", [P, cnt * napp * NAUX], FH, kind="ExternalInput"))
        gidx += cnt
    out_ds, gidx = [], 0
    for gi_, cnt in enumerate(OUTG):
        nb, a = GEOM[gidx]
        napp = (a // 128) * nb
        out_ds.append(nc.dram_tensor(f"outg{gi_}", [P, cnt * N * napp], FH, kind="ExternalOutput"))
        gidx += cnt
    ones_d = nc.dram_tensor("ones", [NB * N, NB], FH, kind="ExternalInput")
    onest_d = nc.dram_tensor("ones_tl", [NB_T * N, NB_T], FH, kind="ExternalInput")

    with TileContext(nc) as tc:
        with (
            nc.allow_low_precision(reason="fp16 pipeline; rel-err gate is 2e-2"),
            tc.tile_pool(name="pw", bufs=2) as pool_w,
            tc.tile_pool(name="pm", bufs=2) as pool_m,
            tc.tile_pool(name="ps", bufs=2) as pool_s,
            tc.tile_pool(name="pones", bufs=1) as pool_c,
            tc.tile_pool(name="ppsum", bufs=3, space="PSUM") as pool_ps,
        ):
            ones = pool_c.tile([NB * N, NB], FH, tag="ones")
            nc.gpsimd.dma_start(out=ones[:], in_=ones_d[:, :])
            ones_t = pool_c.tile([NB_T * N, NB_T], FH, tag="ones_t")
            nc.gpsimd.dma_start(out=ones_t[:], in_=onest_d[:, :])

            pools = (pool_w, pool_m, pool_s, pool_ps)
            specs = [
                (nb, a, (wt_ds[i][:, :], None, None),
                 ones if nb == NB else ones_t, "" if nb == NB else "_t",
                 False)
                for i, (nb, a) in enumerate(GEOM)
            ]
            # per-macro aux slice (grouped loads, issued lazily at first use)
            aux_slices = [None] * len(GEOM)
            aux_tiles = {}
            gidx = 0
            for gi_, cnt in enumerate(AUXG):
                nb, a = GEOM[gidx]
                napp = (a // 128) * nb
                width = napp * NAUX
                for k in range(cnt):
                    aux_slices[gidx + k] = (gi_, k * width, width, cnt * width)
                gidx += cnt
            # per-macro ds slice in a grouped store tile
            out_slices = [None] * len(GEOM)
            gidx = 0
            for gi_, cnt in enumerate(OUTG):
                nb, a = GEOM[gidx]
                napp = (a // 128) * nb
                width = N * napp
                for k in range(cnt):
                    out_slices[gidx + k] = (gi_, k * width, width, cnt * width, k == cnt - 1)
                gidx += cnt
            ds_tiles = {}

            def _get_aux(i):
                gi_, off, width, tot = aux_slices[i]
                if gi_ not in aux_tiles:
                    t = pool_s.tile([P, tot], FH, tag="auxq", bufs=2, name=f"auxg{gi_}")
                    nc.sync.dma_start(out=t[:], in_=aux_ds[gi_][:, :])
                    aux_tiles[gi_] = t
                return aux_tiles[gi_][:, off : off + width]

            def _get_ds(i):
                gi_, off, width, tot, last = out_slices[i]
                if gi_ not in ds_tiles:
                    ds_tiles[gi_] = pool_s.tile([P, tot], FH, tag="dsq", bufs=2, name=f"dsg{gi_}")
                return ds_tiles[gi_][:, off : off + width], (gi_ if last else None)

            loaded = []
            fronted = []
            stores = []
            def _flush_store():
                if stores:
                    gi_ = stores.pop(0)
                    eng = nc.gpsimd if gi_ == len(OUTG) - 1 else nc.sync
                    eng.dma_start(out=out_ds[gi_][:, :], in_=ds_tiles[gi_][:])
            NEG = len(GEOM)
            def _do_back(i, s0, c0):
                # last four backs alternate their chains onto DVE (idle in
                # the drain phase) so consecutive epilogues overlap
                eg = (NEG - 1 - i) % 2 == 1 or i == NEG - 1
                ds_ap, flush_gi = _get_ds(i)
                c0["ds"] = ds_ap
                _back(nc, pools, s0, c0, eg=eg)
                if flush_gi is not None:
                    stores.append(flush_gi)
            bi = 0   # back index
            for i, spec in enumerate(specs):
                loaded.append((spec, _loads(nc, pools, spec, _get_aux(i))))
                _flush_store()
                if len(loaded) == 2:
                    s0, t0 = loaded.pop(0)
                    fronted.append((s0, _front(nc, pools, s0, t0)))
                if len(fronted) == 2:
                    s0, c0 = fronted.pop(0)
                    _do_back(bi, s0, c0)
                    bi += 1
            for s0, t0 in loaded:
                fronted.append((s0, _front(nc, pools, s0, t0)))
            for s0, c0 in fronted:
                _do_back(bi, s0, c0)
                bi += 1
                _flush_store()
            while stores:
                _flush_store()
    if not nc.is_finalized():
        nc.finalize()
    return nc


def make_in_maps(state, w_pos, w_neg, feasibility, perturbation):
    state = np.asarray(state, dtype=np.float32)
    feas = np.asarray(feasibility, dtype=np.float32)
    pert = np.asarray(perturbation, dtype=np.float32)
    s16 = state.astype(np.float16)
    p16 = pert.astype(np.float16)
    a38 = np.concatenate(
        [s16, p16[:, 0:9], p16[:, 13:17], feas.astype(np.float16)], axis=1
    )
    wall = np.stack(
        [np.asarray(w_pos, np.float32), np.asarray(w_neg, np.float32)], axis=1
    ).astype(np.float16)

    in_maps = []
    for c in range(NCORES):
        base = c * B_CORE
        m = {}
        off = 0
        auxes = []
        for i, (nb, a) in enumerate(GEOM):
            gcn = a // 128
            cnt = nb * a
            sl = slice(base + off, base + off + cnt)
            off += cnt
            wm = wall[sl].reshape(nb, gcn, P, 2, N, N)
            wt = np.ascontiguousarray(wm.transpose(0, 5, 3, 4, 1, 2)).reshape(
                nb * N, 2 * N * a
            )
            sm = s16[sl].reshape(nb, gcn, P, N)
            st = np.ascontiguousarray(sm.transpose(0, 3, 1, 2)).reshape(nb * N, a)
            m[f"wt{i}"] = np.concatenate([st, wt], axis=1)
            am = a38[sl].reshape(nb, gcn, P, NAUX)
            auxes.append(
                np.ascontiguousarray(am.transpose(2, 1, 0, 3)).reshape(P, gcn * nb * NAUX)
            )
        gidx = 0
        for gi_, cnt in enumerate(AUXG):
            m[f"auxg{gi_}"] = np.concatenate(auxes[gidx : gidx + cnt], axis=1)
            gidx += cnt
        m["ones"] = np.kron(np.eye(NB, dtype=np.float16), np.ones((N, 1), np.float16))
        m["ones_tl"] = np.kron(np.eye(NB_T, dtype=np.float16), np.ones((N, 1), np.float16))
        in_maps.append(m)
    return in_maps


def gather(results):
    outs = []
    for r in results:
        parts = []
        gidx = 0
        for gi_, cnt in enumerate(OUTG):
            og = np.asarray(r[f"outg{gi_}"])  # [P, cnt * N * napp]
            w = og.shape[1] // cnt
            for k in range(cnt):
                nb, a = GEOM[gidx]
                gcn = a // 128
                o = og[:, k * w : (k + 1) * w].reshape(P, N, gcn, nb)
                parts.append(o.transpose(3, 2, 0, 1).reshape(nb * a, N))
                gidx += 1
        outs.append(np.concatenate(parts, axis=0))
    return np.concatenate(outs, axis=0).astype(np.float32)


def kernel(t=None, state=None, W_pos=None, W_neg=None, feasibility=None, perturbation=None, **_):
    nc = build_program()
    in_maps = make_in_maps(state, W_pos, W_neg, feasibility, perturbation)
    res = run_bass_kernel_spmd(nc, in_maps, list(range(NCORES)))
    return gather(res.results)


if __name__ == "__main__":
    rng = np.random.default_rng(0)
    inputs = {
        "t": rng.standard_normal(1).astype(np.float32),
        "state": rng.random((B_TOTAL, N), dtype=np.float32),
        "W_pos": rng.random((B_TOTAL, N, N), dtype=np.float32),
        "W_neg": rng.random((B_TOTAL, N, N), dtype=np.float32),
        "feasibility": rng.random((B_TOTAL, 4), dtype=np.float32),
        "perturbation": rng.standard_normal((B_TOTAL, N)).astype(np.float32),
    }
    out = kernel(**inputs)
    print(out.shape, out.dtype)
